# revision 47
# baseline (speedup 1.0000x reference)
"""Trainium2 Bass kernel for nn_ConvWithFilter (per-location conv filters).

Computation: out[n, o, h, w] = relu( sum_k unfold(features)[n, l, k] *
filters[n, l, k, o] ),  l = h*W + w,  k in [0, C*3*3) ordered (c, kh, kw).

Strategy: filters dominate traffic (288 MiB f32 -> 144 MiB bf16). Shard
(n, l-quarter) across 8 cores -> 1024 locations/core, 18 MiB of bf16
filter stream per core -> measured DMA floor ~60us/core. Host converts
filters to bf16 and transposes each location's filter matrix to o-major
([O, K]) so each output channel's K=288 row is contiguous.

Primary kernel (KERNEL_STYLE="tsp", _emit_block_pipe): per 128-location
block (locations on SBUF partitions):
  - two chunked filter DMAs (Act group | DVE group) so compute starts
    while the rest of the block streams in
  - DVE tensor_tensor multiplies (2x bf16 mode) into a prod buffer
  - Act engine reduces A_ACT channels (Copy + accum_out, fp32 accum)
  - DVE reduces D_TTR channels with a 5-level binary add tree (2x mode,
    3x cheaper than any accum-style DVE reduce on real HW) + 9-wide
    tensor_reduce
  - one TSP ReLU for all 32 channels, software-pipelined one block late
    so DVE never waits on a same-block Act result (the in-order
    sequencers would otherwise stall the whole pipeline)
Pool/GpSimd is deliberately unused: measured ~1us fixed cost per
instruction and ~0.3 efficiency make any Pool role a net loss.
Per-core: 8 blocks, one batched f32 output DMA at the end.

Measured on real TRN2 (8 cores, repeat-loop slope): ~75-84us/iteration,
vs ~60us DMA-only floor; DVE and Act are both ~9.5us/block busy against
the ~7us DMA slot, so the kernel is compute-bound by those two engines.
"""

import numpy as np
import ml_dtypes

# Problem constants (hardcoded; kernel.py must be self-contained).
N, C, H, W = 2, 32, 64, 64
KSZ = 3
O = 32                 # out channels
K = C * KSZ * KSZ      # 288 contraction length
L = H * W              # 4096 locations
NCORES = 8
LSH = (N * L) // NCORES   # 1024 locations per core
P = 128                   # locations per block (SBUF partitions)
NBLK = LSH // P           # 8 blocks per core

BF16 = ml_dtypes.bfloat16

KERNEL_STYLE = "tsp"   # "tsp" | "split" | "stt"

# Channel split across engines (sum must be O). Column order [Act|Pool|DVE].
A_ACT = 11   # Act-accumulated channels
P_POOL = 0   # Pool-started tree channels (0 = Pool unused)
D_TTR = 21   # DVE full-tree channels

# DMA chunk boundaries (channel counts) for the per-block filter stream.
CHUNKS = (A_ACT, P_POOL, D_TTR)

TRACE = False
TRACE_KW = {}

_CACHE = {}

TREE_WIDTHS = [144, 72, 36, 18, 9]
SCRP_BUFS = 2
FILTP_BUFS = 4
PRODP_BUFS = 3
ACCP_BUFS = 3
SCR_ALT = True
RELU_ON_POOL = True
TREE_SPLIT = False
MULT_MERGE = False
ACT_SCR = "sbuf"   # "sbuf" | "psum" | "fp8"
MULT_SQUARE = False  # diagnostic: ft*ft instead of ft*broadcast(fe)
PROD_PSUM = False    # Act-group product slice in PSUM (off the SBUF ports)
POOL_L5 = False
FE_MONO = False
POOL_LEVELS = 2   # tree levels Pool runs for its channels before DVE takes over


def _build_nc(repeat=1, style=None):
    from concourse import bacc, tile, mybir
    from contextlib import nullcontext

    style = style or KERNEL_STYLE

    nc = bacc.Bacc("TRN2", debug=False)
    dt = mybir.dt

    filt = nc.dram_tensor("filt", [LSH, O * K], dt.bfloat16, kind="ExternalInput")
    feat = nc.dram_tensor("feat", [LSH, K], dt.bfloat16, kind="ExternalInput")
    out = nc.dram_tensor("out", [LSH, O], dt.float32, kind="ExternalOutput")

    filt_ap = filt.ap()
    feat_ap = feat.ap()
    out_ap = out.ap()

    with tile.TileContext(nc) as tc:
        rep_ctx = tc.For_i(0, repeat, 1) if repeat > 1 else nullcontext()
        with (
            tc.tile_pool(name="filtp", bufs={"stt": 6, "tsp": FILTP_BUFS}.get(style, 3)) as filtp,
            tc.tile_pool(name="featp", bufs=2) as featp,
            tc.tile_pool(name="prodp", bufs=PRODP_BUFS) as prodp,
            tc.psum_pool(name="psump", bufs=2) as psump,
            tc.tile_pool(name="scrp", bufs=SCRP_BUFS) as scrp,
            tc.tile_pool(name="accp", bufs=ACCP_BUFS) as accp,
            tc.tile_pool(name="outp", bufs=2) as outp,
            rep_ctx,
        ):
            # Features for this core, DMA'd per block just ahead of use so
            # the first multiply starts as early as possible: [P, NBLK, K]
            fe_all = featp.tile([P, NBLK * K], dt.bfloat16, tag="fe")
            out_all = outp.tile([P, NBLK * O], dt.float32, tag="oa")
            if style.startswith("abl:"):
                # ablation kernels may leave out_all (partially) unwritten
                nc.vector.memset(out_all[:], 0.0)

            if FE_MONO:
                nc.sync.dma_start(
                    out=fe_all[:].rearrange("q (b k) -> q b k", k=K),
                    in_=feat_ap.rearrange("(b q) k -> q b k", q=P),
                )
            for b in range(NBLK):
                rows = slice(b * P, (b + 1) * P)
                if not FE_MONO:
                    nc.sync.dma_start(
                        out=fe_all[:, b * K : (b + 1) * K], in_=feat_ap[rows, :]
                    )
                ft = filtp.tile([P, O * K], dt.bfloat16, tag="ft")
                # Chunked filter stream so compute starts early.
                c0 = 0
                for nch in ((O,) if MULT_MERGE else CHUNKS):
                    if nch == 0:
                        continue
                    c1 = c0 + nch * K
                    nc.sync.dma_start(
                        out=ft[:, c0:c1], in_=filt_ap[rows, c0:c1]
                    )
                    c0 = c1
                fe = fe_all[:, b * K : (b + 1) * K]
                ob = out_all[:, b * O : (b + 1) * O]

                if style == "stt":
                    _emit_block_stt(nc, tc, mybir, dt, scrp, accp, ft, fe, ob)
                elif style == "tsp":
                    carry = _emit_block_pipe(nc, mybir, dt, prodp, scrp, accp,
                                             ft, fe, ob, carry if b else None,
                                             psump=psump)
                elif style.startswith("abl:"):
                    # ablation: "abl:" (DMA only), "abl:mult", "abl:mult,dve", ...
                    parts = tuple(x for x in style[4:].split(",") if x)
                    _emit_block_tsp(nc, tc, mybir, dt, prodp, scrp, accp, ft, fe, ob,
                                    parts=parts)
                else:
                    _emit_block_split(nc, tc, mybir, dt, scrp, accp, ft, fe, ob)

            if style == "tsp":
                _emit_pipe_drain(nc, mybir, dt, scrp, carry)

            nc.sync.dma_start(
                out=out_ap.rearrange("(b q) o -> q b o", q=P),
                in_=out_all[:].rearrange("q (b o) -> q b o", o=O),
            )
    nc.compile()
    return nc


def _emit_block_stt(nc, tc, mybir, dt, scrp, accp, ft, fe, ob):
    """One fused multiply+reduce DVE instruction per output channel."""
    acc = accp.tile([P, O], dt.float32, tag="acc")
    scr = scrp.tile([P, K], dt.bfloat16, tag="scr")
    for o in range(O):
        nc.vector.scalar_tensor_tensor(
            out=scr[:],
            in0=ft[:, o * K : (o + 1) * K],
            scalar=1.0,
            in1=fe,
            op0=mybir.AluOpType.mult,
            op1=mybir.AluOpType.mult,
            accum_out=acc[:, o : o + 1],
        )
    nc.vector.tensor_scalar_max(out=ob[:], in0=acc[:], scalar1=0.0)


def _emit_block_pipe(nc, mybir, dt, prodp, scrp, accp, ft, fe, ob, carry,
                     psump=None):
    """Software-pipelined block: DVE consumes Pool's partial tree and Act's
    accumulators from the PREVIOUS block, so no engine ever waits on a
    same-block cross-engine result (the in-order sequencers would stall).

    Per block b:
      DVE : mults(b) | d-tree(b)+reduce+relu | p-cont(b-1)+reduce+relu |
            relu_a(b-1)
      Act : a accumulates(b)
      Pool: first POOL_LEVELS tree levels for p channels(b)
    Returns carry = (qp, acc_a, ob) for block b; pass the previous carry in.
    """
    a, p, d = A_ACT, P_POOL, D_TTR
    AK, PK = a * K, p * K
    AO = mybir.AluOpType

    def mult(dst, lo, nch):
        nc.vector.tensor_tensor(
            out=dst.rearrange("q (c k) -> q c k", k=K),
            in0=ft[:, lo : lo + nch * K].rearrange("q (c k) -> q c k", k=K),
            in1=fe.broadcast_to((P, K, nch)).rearrange("q k c -> q c k"),
            op=AO.mult,
        )

    if PROD_PSUM and not p:
        # Act-group product in PSUM: Act's full-rate read stream leaves the
        # SBUF ports, decoupling it from DVE's tree traffic.
        pa = psump.tile([P, a * K], dt.bfloat16, tag="pa")
        pd = prodp.tile([P, d * K], dt.bfloat16, tag="pd")
        mult(pa[:], 0, a)
        mult(pd[:], AK, d)
        prod_a, prod_d = pa, pd
    else:
        prod = prodp.tile([P, O * K], dt.bfloat16, tag="prod")
        groups = ((0, O),) if MULT_MERGE else ((0, a), (AK, p), (AK + PK, d))
        for lo, nch in groups:
            if nch:
                mult(prod[:, lo : lo + nch * K], lo, nch)
        prod_a = prod
        prod_d = None

    def tree(eng, cur, nch, seg, levels):
        for li, w in enumerate(levels):
            t = scrp.tile([P, nch * w], dt.bfloat16, tag=f"tr{seg}{li}")
            tv = t[:].rearrange("q (c k) -> q c k", k=w)
            eng.tensor_tensor(out=tv, in0=cur[:, :, :w],
                              in1=cur[:, :, w : 2 * w], op=AO.add)
            cur = tv
        return cur

    # Shared accumulator for the whole block: Act writes cols [0, a),
    # DVE cols [a+p, O); the previous block's Pool channels land in
    # [a, a+p) during the next block. One TSP relu covers all 32.
    acc = accp.tile([P, O], dt.float32, tag="acc")

    # Act: accumulate a channels of this block. The dummy out stream is
    # pure waste; PSUM/fp8 variants cut its SBUF write-port traffic, which
    # contends with DVE's 2x-mode reads (measured as a superlinear cost
    # when Act and the DVE tree run together).
    if ACT_SCR == "psum":
        scr_a0 = scrp.tile([P, K], dt.float32, tag="scr_a0", space="PSUM")
        scr_a1 = scrp.tile([P, K], dt.float32, tag="scr_a1", space="PSUM")
    elif ACT_SCR == "fp8":
        scr_a0 = scrp.tile([P, K], dt.float8e3, tag="scr_a0")
        scr_a1 = scrp.tile([P, K], dt.float8e3, tag="scr_a1")
    else:
        scr_a0 = scrp.tile([P, K], dt.bfloat16, tag="scr_a0")
        scr_a1 = scrp.tile([P, K], dt.bfloat16, tag="scr_a1")
    scr_as = (scr_a0, scr_a1) if SCR_ALT else (scr_a0, scr_a0)
    for i in range(a):
        nc.scalar.activation(
            out=scr_as[i % 2][:],
            in_=prod_a[:, i * K : (i + 1) * K],
            func=mybir.ActivationFunctionType.Copy,
            accum_out=acc[:, i : i + 1],
        )

    # Pool: first POOL_LEVELS levels for p channels of this block
    qp = None
    if p:
        qp = tree(nc.gpsimd,
                  prod_a[:, AK : AK + PK].rearrange("q (c k) -> q c k", k=K),
                  p, "p", TREE_WIDTHS[:POOL_LEVELS])

    # DVE: own d channels of this block (tree + reduce)
    if d and POOL_L5 and not p:
        # DVE runs levels 1-4; the idle Pool engine does level 5 (one TT);
        # the 9-wide reduce + relu complete one block later (no same-block
        # cross-engine waits anywhere).
        t18 = tree(nc.vector,
                   (prod_d[:] if prod_d is not None
                    else prod_a[:, AK + PK :]).rearrange("q (c k) -> q c k", k=K),
                   d, "d", TREE_WIDTHS[:4])
        t9 = scrp.tile([P, d * 9], dt.bfloat16, tag="t9p")
        t9v = t9[:].rearrange("q (c k) -> q c k", k=9)
        nc.gpsimd.tensor_tensor(out=t9v, in0=t18[:, :, :9],
                                in1=t18[:, :, 9:18], op=AO.add)
        if carry is not None:
            _emit_pipe_drain(nc, mybir, dt, scrp, carry)
        return ("l5", t9v, acc, ob)
    if d and TREE_SPLIT:
        # two interleaved independent chains hide the per-level SBUF
        # write-ack latency between data-dependent TT adds
        d1 = (d + 1) // 2
        d2 = d - d1
        pv = (prod_d[:] if prod_d is not None
              else prod_a[:, AK + PK :]).rearrange("q (c k) -> q c k", k=K)
        cur1, cur2 = pv[:, :d1, :], pv[:, d1:, :]
        for li, w in enumerate(TREE_WIDTHS):
            t1 = scrp.tile([P, d1 * w], dt.bfloat16, tag=f"trda{li}")
            v1 = t1[:].rearrange("q (c k) -> q c k", k=w)
            nc.vector.tensor_tensor(out=v1, in0=cur1[:, :, :w],
                                    in1=cur1[:, :, w : 2 * w], op=AO.add)
            t2 = scrp.tile([P, d2 * w], dt.bfloat16, tag=f"trdb{li}")
            v2 = t2[:].rearrange("q (c k) -> q c k", k=w)
            nc.vector.tensor_tensor(out=v2, in0=cur2[:, :, :w],
                                    in1=cur2[:, :, w : 2 * w], op=AO.add)
            cur1, cur2 = v1, v2
        nc.vector.tensor_reduce(out=acc[:, a + p : a + p + d1], in_=cur1,
                                axis=mybir.AxisListType.X, op=AO.add)
        nc.vector.tensor_reduce(out=acc[:, a + p + d1 : O], in_=cur2,
                                axis=mybir.AxisListType.X, op=AO.add)
    elif d:
        t9d = tree(nc.vector,
                   (prod_d[:] if prod_d is not None
                    else prod_a[:, AK + PK :]).rearrange("q (c k) -> q c k", k=K),
                   d, "d", TREE_WIDTHS)
        nc.vector.tensor_reduce(out=acc[:, a + p : O], in_=t9d,
                                axis=mybir.AxisListType.X, op=AO.add)

    # DVE: finish the PREVIOUS block (Pool tail + one relu for all 32)
    if carry is not None:
        _emit_pipe_drain(nc, mybir, dt, scrp, carry)

    return (qp, acc, ob)


def _emit_pipe_drain(nc, mybir, dt, scrp, carry):
    """DVE-side completion of one block: Pool-channel tail + single relu."""
    a, p, d = A_ACT, P_POOL, D_TTR
    AO = mybir.AluOpType
    if carry[0] == "l5":
        _, t9v, acc, ob = carry
        nc.vector.tensor_reduce(out=acc[:, a:O], in_=t9v,
                                axis=mybir.AxisListType.X, op=AO.add)
        eng = nc.gpsimd if RELU_ON_POOL else nc.vector
        eng.tensor_scalar_max(out=ob[:], in0=acc[:], scalar1=0.0)
        return
    qp, acc, ob = carry

    def tree(cur, nch, seg, levels):
        for li, w in enumerate(levels):
            t = scrp.tile([P, nch * w], dt.bfloat16, tag=f"tr{seg}{li}")
            tv = t[:].rearrange("q (c k) -> q c k", k=w)
            nc.vector.tensor_tensor(out=tv, in0=cur[:, :, :w],
                                    in1=cur[:, :, w : 2 * w], op=AO.add)
            cur = tv
        return cur

    if p:
        t9p = tree(qp, p, "pc", TREE_WIDTHS[POOL_LEVELS:])
        nc.vector.tensor_reduce(out=acc[:, a : a + p], in_=t9p,
                                axis=mybir.AxisListType.X, op=AO.add)
    # ReLU on the idle Pool engine frees ~0.2us/block of DVE time; the
    # operands are a block old, so Pool never stalls anything.
    eng = nc.gpsimd if RELU_ON_POOL else nc.vector
    eng.tensor_scalar_max(out=ob[:], in0=acc[:], scalar1=0.0)


def _emit_block_tsp(nc, tc, mybir, dt, prodp, scrp, accp, ft, fe, ob,
                    parts=("mult", "act", "pool", "dve")):
    """DVE multiplies (2x bf16); per-channel reductions via
    tensor_scalar(accum_out) on DVE (4x perf mode), Act Copy-accum, and
    Pool add tree. ReLU on the owning engine (a-group relu on Pool).
    `parts` restricts the emitted compute for HW ablation timing."""
    a, p, d = A_ACT, P_POOL, D_TTR
    AK, PK = a * K, p * K
    AO = mybir.AluOpType

    def mult(prod_view, ft_view, nch):
        if MULT_SQUARE:
            in1 = ft_view
        else:
            in1 = fe.broadcast_to((P, K, nch)).rearrange("q k c -> q c k")
            prod_view = prod_view.rearrange("q (c k) -> q c k", k=K)
            ft_view = ft_view.rearrange("q (c k) -> q c k", k=K)
        nc.vector.tensor_tensor(
            out=prod_view, in0=ft_view, in1=in1, op=AO.mult,
        )

    if "mult" not in parts:
        return
    prod = prodp.tile([P, O * K], dt.bfloat16, tag="prod")
    mult(prod[:, :AK], ft[:, :AK], a)
    if p:
        mult(prod[:, AK : AK + PK], ft[:, AK : AK + PK], p)
    mult(prod[:, AK + PK :], ft[:, AK + PK :], d)

    if "act" in parts:
        # Act accumulates channels [0, a)
        acc_a = accp.tile([P, a], dt.float32, tag="acc_a")
        scr_a0 = scrp.tile([P, K], dt.bfloat16, tag="scr_a0")
        scr_a1 = scrp.tile([P, K], dt.bfloat16, tag="scr_a1")
        scr_as = [scr_a0, scr_a1]
        for i in range(a):
            nc.scalar.activation(
                out=scr_as[i % 2][:],
                in_=prod[:, i * K : (i + 1) * K],
                func=mybir.ActivationFunctionType.Copy,
                accum_out=acc_a[:, i : i + 1],
            )

    def tree(eng, cur, nch, seg, levels):
        """Binary add tree over the last axis of [P, nch, w0]; `levels`
        entries of TREE_WIDTHS starting where w0 = 2*levels[0]."""
        for li, w in enumerate(levels):
            t = scrp.tile([P, nch * w], dt.bfloat16, tag=f"tr{seg}{li}")
            tv = t[:].rearrange("q (c k) -> q c k", k=w)
            eng.tensor_tensor(
                out=tv, in0=cur[:, :, :w], in1=cur[:, :, w : 2 * w],
                op=AO.add,
            )
            cur = tv
        return cur

    if "pool" in parts and p:
        # Pool does the first POOL_LEVELS big tree levels for its channels
        # (gpsimd has ~0.8us fixed cost per instruction on real HW, so
        # minimize Pool instruction count); DVE finishes the tail.
        qp = tree(nc.gpsimd,
                  prod[:, AK : AK + PK].rearrange("q (c k) -> q c k", k=K),
                  p, "p", TREE_WIDTHS[:POOL_LEVELS])

    if "dve" in parts:
        # DVE add tree for channels [a+p, O): TT adds run in 2x bf16 mode
        # on real HW (TSP-accum measured 1x there - tree is 3x cheaper).
        acc_d = accp.tile([P, p + d], dt.float32, tag="acc_d")
        t9d = tree(nc.vector,
                   prod[:, AK + PK :].rearrange("q (c k) -> q c k", k=K),
                   d, "d", TREE_WIDTHS)
        nc.vector.tensor_reduce(out=acc_d[:, p : p + d], in_=t9d,
                                axis=mybir.AxisListType.X, op=AO.add)
        if "pool" in parts and p:
            t9p = tree(nc.vector, qp, p, "pc", TREE_WIDTHS[POOL_LEVELS:])
            nc.vector.tensor_reduce(out=acc_d[:, 0:p], in_=t9p,
                                    axis=mybir.AxisListType.X, op=AO.add)

    # ReLU: p+d channels on DVE (one TSP); a-channels joined onto DVE too
    # (DVE trails Act per block, so the join is usually free; putting it
    # on Pool would couple Pool to Act and serialize the pipeline).
    if "dve" in parts:
        if "pool" in parts and p:
            nc.vector.tensor_scalar_max(out=ob[:, a:O], in0=acc_d[:], scalar1=0.0)
        else:
            nc.vector.tensor_scalar_max(
                out=ob[:, a + p : O], in0=acc_d[:, p : p + d], scalar1=0.0
            )
    if "act" in parts:
        eng = nc.vector if "dve" in parts else nc.gpsimd
        eng.tensor_scalar_max(out=ob[:, 0:a], in0=acc_a[:], scalar1=0.0)


def _emit_block_split(nc, tc, mybir, dt, scrp, accp, ft, fe, ob):
    """3-engine reduction split fallback (DVE mults; Act/Pool/DVE reduce)."""
    a, p, d = A_ACT, P_POOL, D_TTR
    AK, PK = a * K, p * K

    def mult(prod_view, ft_view, nch):
        nc.vector.tensor_tensor(
            out=prod_view.rearrange("q (c k) -> q c k", k=K),
            in0=ft_view.rearrange("q (c k) -> q c k", k=K),
            in1=fe.broadcast_to((P, K, nch)).rearrange("q k c -> q c k"),
            op=mybir.AluOpType.mult,
        )

    def tree(eng, prod_view, nch, seg):
        cur = prod_view
        for li, w in enumerate(TREE_WIDTHS):
            t = scrp.tile([P, nch * w], dt.bfloat16, tag=f"tr{seg}{li}")
            tv = t[:].rearrange("q (c k) -> q c k", k=w)
            eng.tensor_tensor(
                out=tv, in0=cur[:, :, :w], in1=cur[:, :, w : 2 * w],
                op=mybir.AluOpType.add,
            )
            cur = tv
        return cur

    prod = scrp.tile([P, O * K], dt.bfloat16, tag="prod")
    mult(prod[:, :AK], ft[:, :AK], a)
    mult(prod[:, AK : AK + PK], ft[:, AK : AK + PK], p)
    mult(prod[:, AK + PK :], ft[:, AK + PK :], d)

    # Act accumulates channels [0, a)
    acc_a = accp.tile([P, a], dt.float32, tag="acc_a")
    scr_a = scrp.tile([P, K], dt.bfloat16, tag="scr_a")
    for i in range(a):
        nc.scalar.activation(
            out=scr_a[:],
            in_=prod[:, i * K : (i + 1) * K],
            func=mybir.ActivationFunctionType.Copy,
            accum_out=acc_a[:, i : i + 1],
        )

    # Pool tree for channels [a, a+p), finished on Pool
    q9 = tree(nc.gpsimd, prod[:, AK : AK + PK].rearrange("q (c k) -> q c k", k=K),
              p, "p")
    q4 = scrp.tile([P, p * 4], dt.bfloat16, tag="q4")
    q4v = q4[:].rearrange("q (c k) -> q c k", k=4)
    nc.gpsimd.tensor_tensor(out=q4v, in0=q9[:, :, 0:4], in1=q9[:, :, 4:8],
                            op=mybir.AluOpType.add)
    q2 = scrp.tile([P, p * 2], dt.bfloat16, tag="q2")
    q2v = q2[:].rearrange("q (c k) -> q c k", k=2)
    nc.gpsimd.tensor_tensor(out=q2v, in0=q4v[:, :, 0:2], in1=q4v[:, :, 2:4],
                            op=mybir.AluOpType.add)
    q1 = scrp.tile([P, p], dt.bfloat16, tag="q1")
    q1v = q1[:].rearrange("q (c k) -> q c k", k=1)
    nc.gpsimd.tensor_tensor(out=q1v, in0=q2v[:, :, 0:1], in1=q2v[:, :, 1:2],
                            op=mybir.AluOpType.add)
    qa = scrp.tile([P, p], dt.bfloat16, tag="qa")
    qav = qa[:].rearrange("q (c k) -> q c k", k=1)
    nc.gpsimd.tensor_tensor(out=qav, in0=q1v, in1=q9[:, :, 8:9],
                            op=mybir.AluOpType.add)

    # DVE tree for channels [a+p, O)
    acc_d = accp.tile([P, d], dt.float32, tag="acc_d")
    t9 = tree(nc.vector, prod[:, AK + PK :].rearrange("q (c k) -> q c k", k=K),
              d, "d")
    nc.vector.tensor_reduce(out=acc_d[:], in_=t9, axis=mybir.AxisListType.X,
                            op=mybir.AluOpType.add)

    # ReLU on the owning engine (never DVE<-Act/Pool)
    nc.vector.tensor_scalar_max(out=ob[:, a + p : O], in0=acc_d[:], scalar1=0.0)
    nc.gpsimd.tensor_scalar_max(out=ob[:, a : a + p], in0=qa[:], scalar1=0.0)
    nc.gpsimd.tensor_scalar_max(out=ob[:, 0:a], in0=acc_a[:], scalar1=0.0)


def _build_null_nc():
    """Same ExternalInput/Output signature as _build_nc, minimal work.

    Used by test.py to subtract input-upload + dispatch overhead from the
    wall-clock SPMD time (no NTFF profiling hook under this axon build).
    """
    from concourse import bacc, tile, mybir

    nc = bacc.Bacc("TRN2", debug=False)
    dt = mybir.dt

    nc.dram_tensor("filt", [LSH, O * K], dt.bfloat16, kind="ExternalInput")
    feat = nc.dram_tensor("feat", [LSH, K], dt.bfloat16, kind="ExternalInput")
    out = nc.dram_tensor("out", [LSH, O], dt.float32, kind="ExternalOutput")

    with tile.TileContext(nc) as tc:
        with tc.tile_pool(name="np_", bufs=1) as pool:
            t = pool.tile([P, O], dt.float32, tag="t")
            nc.vector.memset(t[:], 0.0)
            nc.sync.dma_start(out=out.ap()[0:P, :], in_=t[:])
            # touch feat so the input isn't pruned
            tf = pool.tile([P, 8], dt.bfloat16, tag="tf")
            nc.sync.dma_start(out=tf[:], in_=feat.ap()[0:P, 0:8])
    nc.compile()
    return nc


def _unfold_np(x):
    """numpy mirror of the reference unfold: [N,C,H,W] -> [N, L, C*9]."""
    xp = np.pad(x, ((0, 0), (0, 0), (1, 1), (1, 1)))
    patches = [
        xp[:, :, i : i + H, j : j + W] for i in range(KSZ) for j in range(KSZ)
    ]
    unf = np.stack(patches, axis=2)          # [N, C, 9, H, W]
    unf = unf.reshape(N, K, L)               # k = c*9 + (kh*3+kw)
    return unf.transpose(0, 2, 1)            # [N, L, K]


def kernel(features: np.ndarray, filters: np.ndarray) -> np.ndarray:
    from concourse.bass_utils import run_bass_kernel_spmd

    features = np.asarray(features, dtype=np.float32)
    filters = np.asarray(filters, dtype=np.float32)

    feat_unf = _unfold_np(features)          # [N, L, K] f32
    filt_bf = filters.astype(BF16)           # [N, L, K, O]

    in_maps = []
    for core in range(NCORES):
        n, q = divmod(core, NCORES // N)
        sl = slice(q * LSH, (q + 1) * LSH)
        fe = np.ascontiguousarray(feat_unf[n, sl]).astype(BF16)
        # o-major: per location, filter matrix transposed to [O, K]
        ftT = np.ascontiguousarray(
            filt_bf[n, sl].transpose(0, 2, 1)
        ).reshape(LSH, O * K)
        in_maps.append({"filt": ftT, "feat": fe})

    if "nc" not in _CACHE:
        _CACHE["nc"] = _build_nc()
    _CACHE["in_maps"] = in_maps
    res = run_bass_kernel_spmd(
        _CACHE["nc"], in_maps, list(range(NCORES)), trace=TRACE, **TRACE_KW
    )
    _CACHE["last_result"] = res

    out = np.empty((N, O, H, W), np.float32)
    out_flat = out.reshape(N, O, L)
    for core in range(NCORES):
        n, q = divmod(core, NCORES // N)
        o = np.asarray(res.results[core]["out"], dtype=np.float32)  # [LSH, O]
        out_flat[n, :, q * LSH : (q + 1) * LSH] = o.T
    return out


# revision 48
# speedup vs baseline: 1.0469x; 1.0469x over previous
"""Trainium2 Bass kernel for nn_ConvWithFilter (per-location conv filters).

Computation: out[n, o, h, w] = relu( sum_k unfold(features)[n, l, k] *
filters[n, l, k, o] ),  l = h*W + w,  k in [0, C*3*3) ordered (c, kh, kw).

Strategy: filters dominate traffic (288 MiB f32 -> 144 MiB bf16). Shard
(n, l-quarter) across 8 cores -> 1024 locations/core, 18 MiB of bf16
filter stream per core -> measured DMA floor ~60us/core. Host converts
filters to bf16 and transposes each location's filter matrix to o-major
([O, K]) so each output channel's K=288 row is contiguous.

Primary kernel (KERNEL_STYLE="tsp", _emit_block_pipe): per 128-location
block (locations on SBUF partitions):
  - two chunked filter DMAs (Act group | DVE group) so compute starts
    while the rest of the block streams in
  - DVE tensor_tensor multiplies (2x bf16 mode) into a prod buffer
  - Act engine reduces A_ACT channels (Copy + accum_out, fp32 accum)
  - DVE reduces D_TTR channels with a 5-level binary add tree (2x mode,
    3x cheaper than any accum-style DVE reduce on real HW) + 9-wide
    tensor_reduce
  - one TSP ReLU for all 32 channels, software-pipelined one block late
    so DVE never waits on a same-block Act result (the in-order
    sequencers would otherwise stall the whole pipeline)
Pool/GpSimd is deliberately unused: measured ~1us fixed cost per
instruction and ~0.3 efficiency make any Pool role a net loss.
Per-core: 8 blocks, one batched f32 output DMA at the end.

Measured on real TRN2 (8 cores, repeat-loop slope): ~75-84us/iteration,
vs ~60us DMA-only floor; DVE and Act are both ~9.5us/block busy against
the ~7us DMA slot, so the kernel is compute-bound by those two engines.
"""

import numpy as np
import ml_dtypes

# Problem constants (hardcoded; kernel.py must be self-contained).
N, C, H, W = 2, 32, 64, 64
KSZ = 3
O = 32                 # out channels
K = C * KSZ * KSZ      # 288 contraction length
L = H * W              # 4096 locations
NCORES = 8
LSH = (N * L) // NCORES   # 1024 locations per core
P = 128                   # locations per block (SBUF partitions)
NBLK = LSH // P           # 8 blocks per core

BF16 = ml_dtypes.bfloat16

KERNEL_STYLE = "tsp"   # "tsp" | "split" | "stt"

# Channel split across engines (sum must be O). Column order [Act|Pool|DVE].
A_ACT = 11   # Act-accumulated channels
P_POOL = 0   # Pool-started tree channels (0 = Pool unused)
D_TTR = 21   # DVE full-tree channels

# DMA chunk boundaries (channel counts) for the per-block filter stream.
CHUNKS = (A_ACT, P_POOL, D_TTR)

TRACE = False
TRACE_KW = {}

_CACHE = {}

TREE_WIDTHS = [144, 72, 36, 18, 9]
SCRP_BUFS = 2
FILTP_BUFS = 4
PRODP_BUFS = 3
ACCP_BUFS = 3
SCR_ALT = True
RELU_ON_POOL = True
TREE_SPLIT = False
MULT_MERGE = False
ACT_SCR = "sbuf"   # "sbuf" | "psum" | "fp8"
MULT_SQUARE = False  # diagnostic: ft*ft instead of ft*broadcast(fe)
PROD_PSUM = False    # Act-group product slice in PSUM (off the SBUF ports)
SCRP_SIDE = None     # None | "right": DVE tree scratch at opposite SBUF end
POOL_L5 = False
FE_MONO = False
POOL_LEVELS = 2   # tree levels Pool runs for its channels before DVE takes over


def _build_nc(repeat=1, style=None):
    from concourse import bacc, tile, mybir
    from contextlib import nullcontext

    style = style or KERNEL_STYLE

    nc = bacc.Bacc("TRN2", debug=False)
    dt = mybir.dt

    filt = nc.dram_tensor("filt", [LSH, O * K], dt.bfloat16, kind="ExternalInput")
    feat = nc.dram_tensor("feat", [LSH, K], dt.bfloat16, kind="ExternalInput")
    out = nc.dram_tensor("out", [LSH, O], dt.float32, kind="ExternalOutput")

    filt_ap = filt.ap()
    feat_ap = feat.ap()
    out_ap = out.ap()

    with tile.TileContext(nc) as tc:
        rep_ctx = tc.For_i(0, repeat, 1) if repeat > 1 else nullcontext()
        with (
            tc.tile_pool(name="filtp", bufs={"stt": 6, "tsp": FILTP_BUFS}.get(style, 3)) as filtp,
            tc.tile_pool(name="featp", bufs=2) as featp,
            tc.tile_pool(name="prodp", bufs=PRODP_BUFS) as prodp,
            tc.psum_pool(name="psump", bufs=2) as psump,
            tc.tile_pool(name="scrp", bufs=SCRP_BUFS, side=SCRP_SIDE) as scrp,
            tc.tile_pool(name="accp", bufs=ACCP_BUFS) as accp,
            tc.tile_pool(name="outp", bufs=2) as outp,
            rep_ctx,
        ):
            # Features for this core, DMA'd per block just ahead of use so
            # the first multiply starts as early as possible: [P, NBLK, K]
            fe_all = featp.tile([P, NBLK * K], dt.bfloat16, tag="fe")
            out_all = outp.tile([P, NBLK * O], dt.float32, tag="oa")
            if style.startswith("abl:"):
                # ablation kernels may leave out_all (partially) unwritten
                nc.vector.memset(out_all[:], 0.0)

            if FE_MONO:
                nc.sync.dma_start(
                    out=fe_all[:].rearrange("q (b k) -> q b k", k=K),
                    in_=feat_ap.rearrange("(b q) k -> q b k", q=P),
                )
            for b in range(NBLK):
                rows = slice(b * P, (b + 1) * P)
                if not FE_MONO:
                    nc.sync.dma_start(
                        out=fe_all[:, b * K : (b + 1) * K], in_=feat_ap[rows, :]
                    )
                ft = filtp.tile([P, O * K], dt.bfloat16, tag="ft")
                # Chunked filter stream so compute starts early.
                c0 = 0
                for nch in ((O,) if MULT_MERGE else CHUNKS):
                    if nch == 0:
                        continue
                    c1 = c0 + nch * K
                    nc.sync.dma_start(
                        out=ft[:, c0:c1], in_=filt_ap[rows, c0:c1]
                    )
                    c0 = c1
                fe = fe_all[:, b * K : (b + 1) * K]
                ob = out_all[:, b * O : (b + 1) * O]

                if style == "stt":
                    _emit_block_stt(nc, tc, mybir, dt, scrp, accp, ft, fe, ob)
                elif style == "tsp":
                    carry = _emit_block_pipe(nc, mybir, dt, prodp, scrp, accp,
                                             ft, fe, ob, carry if b else None,
                                             psump=psump)
                elif style.startswith("abl:"):
                    # ablation: "abl:" (DMA only), "abl:mult", "abl:mult,dve", ...
                    parts = tuple(x for x in style[4:].split(",") if x)
                    _emit_block_tsp(nc, tc, mybir, dt, prodp, scrp, accp, ft, fe, ob,
                                    parts=parts)
                else:
                    _emit_block_split(nc, tc, mybir, dt, scrp, accp, ft, fe, ob)

            if style == "tsp":
                _emit_pipe_drain(nc, mybir, dt, scrp, carry)

            nc.sync.dma_start(
                out=out_ap.rearrange("(b q) o -> q b o", q=P),
                in_=out_all[:].rearrange("q (b o) -> q b o", o=O),
            )
    nc.compile()
    return nc


def _emit_block_stt(nc, tc, mybir, dt, scrp, accp, ft, fe, ob):
    """One fused multiply+reduce DVE instruction per output channel."""
    acc = accp.tile([P, O], dt.float32, tag="acc")
    scr = scrp.tile([P, K], dt.bfloat16, tag="scr")
    for o in range(O):
        nc.vector.scalar_tensor_tensor(
            out=scr[:],
            in0=ft[:, o * K : (o + 1) * K],
            scalar=1.0,
            in1=fe,
            op0=mybir.AluOpType.mult,
            op1=mybir.AluOpType.mult,
            accum_out=acc[:, o : o + 1],
        )
    nc.vector.tensor_scalar_max(out=ob[:], in0=acc[:], scalar1=0.0)


def _emit_block_pipe(nc, mybir, dt, prodp, scrp, accp, ft, fe, ob, carry,
                     psump=None):
    """Software-pipelined block: DVE consumes Pool's partial tree and Act's
    accumulators from the PREVIOUS block, so no engine ever waits on a
    same-block cross-engine result (the in-order sequencers would stall).

    Per block b:
      DVE : mults(b) | d-tree(b)+reduce+relu | p-cont(b-1)+reduce+relu |
            relu_a(b-1)
      Act : a accumulates(b)
      Pool: first POOL_LEVELS tree levels for p channels(b)
    Returns carry = (qp, acc_a, ob) for block b; pass the previous carry in.
    """
    a, p, d = A_ACT, P_POOL, D_TTR
    AK, PK = a * K, p * K
    AO = mybir.AluOpType

    def mult(dst, lo, nch):
        nc.vector.tensor_tensor(
            out=dst.rearrange("q (c k) -> q c k", k=K),
            in0=ft[:, lo : lo + nch * K].rearrange("q (c k) -> q c k", k=K),
            in1=fe.broadcast_to((P, K, nch)).rearrange("q k c -> q c k"),
            op=AO.mult,
        )

    if PROD_PSUM and not p:
        # Act-group product in PSUM: Act's full-rate read stream leaves the
        # SBUF ports, decoupling it from DVE's tree traffic.
        pa = psump.tile([P, a * K], dt.bfloat16, tag="pa")
        pd = prodp.tile([P, d * K], dt.bfloat16, tag="pd")
        mult(pa[:], 0, a)
        mult(pd[:], AK, d)
        prod_a, prod_d = pa, pd
    else:
        prod = prodp.tile([P, O * K], dt.bfloat16, tag="prod")
        groups = ((0, O),) if MULT_MERGE else ((0, a), (AK, p), (AK + PK, d))
        for lo, nch in groups:
            if nch:
                mult(prod[:, lo : lo + nch * K], lo, nch)
        prod_a = prod
        prod_d = None

    def tree(eng, cur, nch, seg, levels):
        for li, w in enumerate(levels):
            t = scrp.tile([P, nch * w], dt.bfloat16, tag=f"tr{seg}{li}")
            tv = t[:].rearrange("q (c k) -> q c k", k=w)
            eng.tensor_tensor(out=tv, in0=cur[:, :, :w],
                              in1=cur[:, :, w : 2 * w], op=AO.add)
            cur = tv
        return cur

    # Shared accumulator for the whole block: Act writes cols [0, a),
    # DVE cols [a+p, O); the previous block's Pool channels land in
    # [a, a+p) during the next block. One TSP relu covers all 32.
    acc = accp.tile([P, O], dt.float32, tag="acc")

    # Act: accumulate a channels of this block. The dummy out stream is
    # pure waste; PSUM/fp8 variants cut its SBUF write-port traffic, which
    # contends with DVE's 2x-mode reads (measured as a superlinear cost
    # when Act and the DVE tree run together).
    if ACT_SCR == "psum":
        scr_a0 = scrp.tile([P, K], dt.float32, tag="scr_a0", space="PSUM")
        scr_a1 = scrp.tile([P, K], dt.float32, tag="scr_a1", space="PSUM")
    elif ACT_SCR == "fp8":
        scr_a0 = scrp.tile([P, K], dt.float8e3, tag="scr_a0")
        scr_a1 = scrp.tile([P, K], dt.float8e3, tag="scr_a1")
    else:
        scr_a0 = scrp.tile([P, K], dt.bfloat16, tag="scr_a0")
        scr_a1 = scrp.tile([P, K], dt.bfloat16, tag="scr_a1")
    scr_as = (scr_a0, scr_a1) if SCR_ALT else (scr_a0, scr_a0)
    for i in range(a):
        nc.scalar.activation(
            out=scr_as[i % 2][:],
            in_=prod_a[:, i * K : (i + 1) * K],
            func=mybir.ActivationFunctionType.Copy,
            accum_out=acc[:, i : i + 1],
        )

    # Pool: first POOL_LEVELS levels for p channels of this block
    qp = None
    if p:
        qp = tree(nc.gpsimd,
                  prod_a[:, AK : AK + PK].rearrange("q (c k) -> q c k", k=K),
                  p, "p", TREE_WIDTHS[:POOL_LEVELS])

    # DVE: own d channels of this block (tree + reduce)
    if d and POOL_L5 and not p:
        # DVE runs levels 1-4; the idle Pool engine does level 5 (one TT);
        # the 9-wide reduce + relu complete one block later (no same-block
        # cross-engine waits anywhere).
        t18 = tree(nc.vector,
                   (prod_d[:] if prod_d is not None
                    else prod_a[:, AK + PK :]).rearrange("q (c k) -> q c k", k=K),
                   d, "d", TREE_WIDTHS[:4])
        t9 = scrp.tile([P, d * 9], dt.bfloat16, tag="t9p")
        t9v = t9[:].rearrange("q (c k) -> q c k", k=9)
        nc.gpsimd.tensor_tensor(out=t9v, in0=t18[:, :, :9],
                                in1=t18[:, :, 9:18], op=AO.add)
        if carry is not None:
            _emit_pipe_drain(nc, mybir, dt, scrp, carry)
        return ("l5", t9v, acc, ob)
    if d and TREE_SPLIT:
        # two interleaved independent chains hide the per-level SBUF
        # write-ack latency between data-dependent TT adds
        d1 = (d + 1) // 2
        d2 = d - d1
        pv = (prod_d[:] if prod_d is not None
              else prod_a[:, AK + PK :]).rearrange("q (c k) -> q c k", k=K)
        cur1, cur2 = pv[:, :d1, :], pv[:, d1:, :]
        for li, w in enumerate(TREE_WIDTHS):
            t1 = scrp.tile([P, d1 * w], dt.bfloat16, tag=f"trda{li}")
            v1 = t1[:].rearrange("q (c k) -> q c k", k=w)
            nc.vector.tensor_tensor(out=v1, in0=cur1[:, :, :w],
                                    in1=cur1[:, :, w : 2 * w], op=AO.add)
            t2 = scrp.tile([P, d2 * w], dt.bfloat16, tag=f"trdb{li}")
            v2 = t2[:].rearrange("q (c k) -> q c k", k=w)
            nc.vector.tensor_tensor(out=v2, in0=cur2[:, :, :w],
                                    in1=cur2[:, :, w : 2 * w], op=AO.add)
            cur1, cur2 = v1, v2
        nc.vector.tensor_reduce(out=acc[:, a + p : a + p + d1], in_=cur1,
                                axis=mybir.AxisListType.X, op=AO.add)
        nc.vector.tensor_reduce(out=acc[:, a + p + d1 : O], in_=cur2,
                                axis=mybir.AxisListType.X, op=AO.add)
    elif d:
        t9d = tree(nc.vector,
                   (prod_d[:] if prod_d is not None
                    else prod_a[:, AK + PK :]).rearrange("q (c k) -> q c k", k=K),
                   d, "d", TREE_WIDTHS)
        nc.vector.tensor_reduce(out=acc[:, a + p : O], in_=t9d,
                                axis=mybir.AxisListType.X, op=AO.add)

    # DVE: finish the PREVIOUS block (Pool tail + one relu for all 32)
    if carry is not None:
        _emit_pipe_drain(nc, mybir, dt, scrp, carry)

    return (qp, acc, ob)


def _emit_pipe_drain(nc, mybir, dt, scrp, carry):
    """DVE-side completion of one block: Pool-channel tail + single relu."""
    a, p, d = A_ACT, P_POOL, D_TTR
    AO = mybir.AluOpType
    if carry[0] == "l5":
        _, t9v, acc, ob = carry
        nc.vector.tensor_reduce(out=acc[:, a:O], in_=t9v,
                                axis=mybir.AxisListType.X, op=AO.add)
        eng = nc.gpsimd if RELU_ON_POOL else nc.vector
        eng.tensor_scalar_max(out=ob[:], in0=acc[:], scalar1=0.0)
        return
    qp, acc, ob = carry

    def tree(cur, nch, seg, levels):
        for li, w in enumerate(levels):
            t = scrp.tile([P, nch * w], dt.bfloat16, tag=f"tr{seg}{li}")
            tv = t[:].rearrange("q (c k) -> q c k", k=w)
            nc.vector.tensor_tensor(out=tv, in0=cur[:, :, :w],
                                    in1=cur[:, :, w : 2 * w], op=AO.add)
            cur = tv
        return cur

    if p:
        t9p = tree(qp, p, "pc", TREE_WIDTHS[POOL_LEVELS:])
        nc.vector.tensor_reduce(out=acc[:, a : a + p], in_=t9p,
                                axis=mybir.AxisListType.X, op=AO.add)
    # ReLU on the idle Pool engine frees ~0.2us/block of DVE time; the
    # operands are a block old, so Pool never stalls anything.
    eng = nc.gpsimd if RELU_ON_POOL else nc.vector
    eng.tensor_scalar_max(out=ob[:], in0=acc[:], scalar1=0.0)


def _emit_block_tsp(nc, tc, mybir, dt, prodp, scrp, accp, ft, fe, ob,
                    parts=("mult", "act", "pool", "dve")):
    """DVE multiplies (2x bf16); per-channel reductions via
    tensor_scalar(accum_out) on DVE (4x perf mode), Act Copy-accum, and
    Pool add tree. ReLU on the owning engine (a-group relu on Pool).
    `parts` restricts the emitted compute for HW ablation timing."""
    a, p, d = A_ACT, P_POOL, D_TTR
    AK, PK = a * K, p * K
    AO = mybir.AluOpType

    def mult(prod_view, ft_view, nch):
        if MULT_SQUARE:
            in1 = ft_view
        else:
            in1 = fe.broadcast_to((P, K, nch)).rearrange("q k c -> q c k")
            prod_view = prod_view.rearrange("q (c k) -> q c k", k=K)
            ft_view = ft_view.rearrange("q (c k) -> q c k", k=K)
        nc.vector.tensor_tensor(
            out=prod_view, in0=ft_view, in1=in1, op=AO.mult,
        )

    if "mult" not in parts:
        return
    prod = prodp.tile([P, O * K], dt.bfloat16, tag="prod")
    mult(prod[:, :AK], ft[:, :AK], a)
    if p:
        mult(prod[:, AK : AK + PK], ft[:, AK : AK + PK], p)
    mult(prod[:, AK + PK :], ft[:, AK + PK :], d)

    if "act" in parts:
        # Act accumulates channels [0, a)
        acc_a = accp.tile([P, a], dt.float32, tag="acc_a")
        scr_a0 = scrp.tile([P, K], dt.bfloat16, tag="scr_a0")
        scr_a1 = scrp.tile([P, K], dt.bfloat16, tag="scr_a1")
        scr_as = [scr_a0, scr_a1]
        for i in range(a):
            nc.scalar.activation(
                out=scr_as[i % 2][:],
                in_=prod[:, i * K : (i + 1) * K],
                func=mybir.ActivationFunctionType.Copy,
                accum_out=acc_a[:, i : i + 1],
            )

    def tree(eng, cur, nch, seg, levels):
        """Binary add tree over the last axis of [P, nch, w0]; `levels`
        entries of TREE_WIDTHS starting where w0 = 2*levels[0]."""
        for li, w in enumerate(levels):
            t = scrp.tile([P, nch * w], dt.bfloat16, tag=f"tr{seg}{li}")
            tv = t[:].rearrange("q (c k) -> q c k", k=w)
            eng.tensor_tensor(
                out=tv, in0=cur[:, :, :w], in1=cur[:, :, w : 2 * w],
                op=AO.add,
            )
            cur = tv
        return cur

    if "pool" in parts and p:
        # Pool does the first POOL_LEVELS big tree levels for its channels
        # (gpsimd has ~0.8us fixed cost per instruction on real HW, so
        # minimize Pool instruction count); DVE finishes the tail.
        qp = tree(nc.gpsimd,
                  prod[:, AK : AK + PK].rearrange("q (c k) -> q c k", k=K),
                  p, "p", TREE_WIDTHS[:POOL_LEVELS])

    if "dve" in parts:
        # DVE add tree for channels [a+p, O): TT adds run in 2x bf16 mode
        # on real HW (TSP-accum measured 1x there - tree is 3x cheaper).
        acc_d = accp.tile([P, p + d], dt.float32, tag="acc_d")
        t9d = tree(nc.vector,
                   prod[:, AK + PK :].rearrange("q (c k) -> q c k", k=K),
                   d, "d", TREE_WIDTHS)
        nc.vector.tensor_reduce(out=acc_d[:, p : p + d], in_=t9d,
                                axis=mybir.AxisListType.X, op=AO.add)
        if "pool" in parts and p:
            t9p = tree(nc.vector, qp, p, "pc", TREE_WIDTHS[POOL_LEVELS:])
            nc.vector.tensor_reduce(out=acc_d[:, 0:p], in_=t9p,
                                    axis=mybir.AxisListType.X, op=AO.add)

    # ReLU: p+d channels on DVE (one TSP); a-channels joined onto DVE too
    # (DVE trails Act per block, so the join is usually free; putting it
    # on Pool would couple Pool to Act and serialize the pipeline).
    if "dve" in parts:
        if "pool" in parts and p:
            nc.vector.tensor_scalar_max(out=ob[:, a:O], in0=acc_d[:], scalar1=0.0)
        else:
            nc.vector.tensor_scalar_max(
                out=ob[:, a + p : O], in0=acc_d[:, p : p + d], scalar1=0.0
            )
    if "act" in parts:
        eng = nc.vector if "dve" in parts else nc.gpsimd
        eng.tensor_scalar_max(out=ob[:, 0:a], in0=acc_a[:], scalar1=0.0)


def _emit_block_split(nc, tc, mybir, dt, scrp, accp, ft, fe, ob):
    """3-engine reduction split fallback (DVE mults; Act/Pool/DVE reduce)."""
    a, p, d = A_ACT, P_POOL, D_TTR
    AK, PK = a * K, p * K

    def mult(prod_view, ft_view, nch):
        nc.vector.tensor_tensor(
            out=prod_view.rearrange("q (c k) -> q c k", k=K),
            in0=ft_view.rearrange("q (c k) -> q c k", k=K),
            in1=fe.broadcast_to((P, K, nch)).rearrange("q k c -> q c k"),
            op=mybir.AluOpType.mult,
        )

    def tree(eng, prod_view, nch, seg):
        cur = prod_view
        for li, w in enumerate(TREE_WIDTHS):
            t = scrp.tile([P, nch * w], dt.bfloat16, tag=f"tr{seg}{li}")
            tv = t[:].rearrange("q (c k) -> q c k", k=w)
            eng.tensor_tensor(
                out=tv, in0=cur[:, :, :w], in1=cur[:, :, w : 2 * w],
                op=mybir.AluOpType.add,
            )
            cur = tv
        return cur

    prod = scrp.tile([P, O * K], dt.bfloat16, tag="prod")
    mult(prod[:, :AK], ft[:, :AK], a)
    mult(prod[:, AK : AK + PK], ft[:, AK : AK + PK], p)
    mult(prod[:, AK + PK :], ft[:, AK + PK :], d)

    # Act accumulates channels [0, a)
    acc_a = accp.tile([P, a], dt.float32, tag="acc_a")
    scr_a = scrp.tile([P, K], dt.bfloat16, tag="scr_a")
    for i in range(a):
        nc.scalar.activation(
            out=scr_a[:],
            in_=prod[:, i * K : (i + 1) * K],
            func=mybir.ActivationFunctionType.Copy,
            accum_out=acc_a[:, i : i + 1],
        )

    # Pool tree for channels [a, a+p), finished on Pool
    q9 = tree(nc.gpsimd, prod[:, AK : AK + PK].rearrange("q (c k) -> q c k", k=K),
              p, "p")
    q4 = scrp.tile([P, p * 4], dt.bfloat16, tag="q4")
    q4v = q4[:].rearrange("q (c k) -> q c k", k=4)
    nc.gpsimd.tensor_tensor(out=q4v, in0=q9[:, :, 0:4], in1=q9[:, :, 4:8],
                            op=mybir.AluOpType.add)
    q2 = scrp.tile([P, p * 2], dt.bfloat16, tag="q2")
    q2v = q2[:].rearrange("q (c k) -> q c k", k=2)
    nc.gpsimd.tensor_tensor(out=q2v, in0=q4v[:, :, 0:2], in1=q4v[:, :, 2:4],
                            op=mybir.AluOpType.add)
    q1 = scrp.tile([P, p], dt.bfloat16, tag="q1")
    q1v = q1[:].rearrange("q (c k) -> q c k", k=1)
    nc.gpsimd.tensor_tensor(out=q1v, in0=q2v[:, :, 0:1], in1=q2v[:, :, 1:2],
                            op=mybir.AluOpType.add)
    qa = scrp.tile([P, p], dt.bfloat16, tag="qa")
    qav = qa[:].rearrange("q (c k) -> q c k", k=1)
    nc.gpsimd.tensor_tensor(out=qav, in0=q1v, in1=q9[:, :, 8:9],
                            op=mybir.AluOpType.add)

    # DVE tree for channels [a+p, O)
    acc_d = accp.tile([P, d], dt.float32, tag="acc_d")
    t9 = tree(nc.vector, prod[:, AK + PK :].rearrange("q (c k) -> q c k", k=K),
              d, "d")
    nc.vector.tensor_reduce(out=acc_d[:], in_=t9, axis=mybir.AxisListType.X,
                            op=mybir.AluOpType.add)

    # ReLU on the owning engine (never DVE<-Act/Pool)
    nc.vector.tensor_scalar_max(out=ob[:, a + p : O], in0=acc_d[:], scalar1=0.0)
    nc.gpsimd.tensor_scalar_max(out=ob[:, a : a + p], in0=qa[:], scalar1=0.0)
    nc.gpsimd.tensor_scalar_max(out=ob[:, 0:a], in0=acc_a[:], scalar1=0.0)


def _build_null_nc():
    """Same ExternalInput/Output signature as _build_nc, minimal work.

    Used by test.py to subtract input-upload + dispatch overhead from the
    wall-clock SPMD time (no NTFF profiling hook under this axon build).
    """
    from concourse import bacc, tile, mybir

    nc = bacc.Bacc("TRN2", debug=False)
    dt = mybir.dt

    nc.dram_tensor("filt", [LSH, O * K], dt.bfloat16, kind="ExternalInput")
    feat = nc.dram_tensor("feat", [LSH, K], dt.bfloat16, kind="ExternalInput")
    out = nc.dram_tensor("out", [LSH, O], dt.float32, kind="ExternalOutput")

    with tile.TileContext(nc) as tc:
        with tc.tile_pool(name="np_", bufs=1) as pool:
            t = pool.tile([P, O], dt.float32, tag="t")
            nc.vector.memset(t[:], 0.0)
            nc.sync.dma_start(out=out.ap()[0:P, :], in_=t[:])
            # touch feat so the input isn't pruned
            tf = pool.tile([P, 8], dt.bfloat16, tag="tf")
            nc.sync.dma_start(out=tf[:], in_=feat.ap()[0:P, 0:8])
    nc.compile()
    return nc


def _unfold_np(x):
    """numpy mirror of the reference unfold: [N,C,H,W] -> [N, L, C*9]."""
    xp = np.pad(x, ((0, 0), (0, 0), (1, 1), (1, 1)))
    patches = [
        xp[:, :, i : i + H, j : j + W] for i in range(KSZ) for j in range(KSZ)
    ]
    unf = np.stack(patches, axis=2)          # [N, C, 9, H, W]
    unf = unf.reshape(N, K, L)               # k = c*9 + (kh*3+kw)
    return unf.transpose(0, 2, 1)            # [N, L, K]


def kernel(features: np.ndarray, filters: np.ndarray) -> np.ndarray:
    from concourse.bass_utils import run_bass_kernel_spmd

    features = np.asarray(features, dtype=np.float32)
    filters = np.asarray(filters, dtype=np.float32)

    feat_unf = _unfold_np(features)          # [N, L, K] f32
    filt_bf = filters.astype(BF16)           # [N, L, K, O]

    in_maps = []
    for core in range(NCORES):
        n, q = divmod(core, NCORES // N)
        sl = slice(q * LSH, (q + 1) * LSH)
        fe = np.ascontiguousarray(feat_unf[n, sl]).astype(BF16)
        # o-major: per location, filter matrix transposed to [O, K]
        ftT = np.ascontiguousarray(
            filt_bf[n, sl].transpose(0, 2, 1)
        ).reshape(LSH, O * K)
        in_maps.append({"filt": ftT, "feat": fe})

    if "nc" not in _CACHE:
        _CACHE["nc"] = _build_nc()
    _CACHE["in_maps"] = in_maps
    res = run_bass_kernel_spmd(
        _CACHE["nc"], in_maps, list(range(NCORES)), trace=TRACE, **TRACE_KW
    )
    _CACHE["last_result"] = res

    out = np.empty((N, O, H, W), np.float32)
    out_flat = out.reshape(N, O, L)
    for core in range(NCORES):
        n, q = divmod(core, NCORES // N)
        o = np.asarray(res.results[core]["out"], dtype=np.float32)  # [LSH, O]
        out_flat[n, :, q * LSH : (q + 1) * LSH] = o.T
    return out
